# revision 8
# baseline (speedup 1.0000x reference)
"""Trainium2 Bass kernel for the EulerIntegrator problem.

Math
----
Reference per step (k = 0..steps-1), dt = 0.01:
    p_k   = v_k @ U                      [B, R]
    q_k   = p_k * p_k
    Gamma = q_k @ W                      [B, D]
    x_{k+1} = x_k + dt * v_k
    v_{k+1} = v_k + dt * (F - Gamma)

Everything is linear except q = p^2, so the whole scan collapses into the
small R-space: with c = dt * (F @ U) and H = dt * (W @ U)  [R, R],
    p_{k+1} = p_k + c - q_k @ H
and the outputs only need the plain / weighted sums of the q_k:
    v_out = v + steps*dt*F - dt * (S @ W),            S = sum_k q_k
    x_out = x + steps*dt*v + C2*dt^2*F - dt^2*(T @ W), T = sum_{k<steps-1} (steps-1-k) q_k
with C2 = steps*(steps-1)/2.

Device layout (per core, batch-sharded 4096/8 = 512):
  All big tensors live transposed, [feature, batch] = [128-partition tiles, 512].
  p, S, T accumulate in PSUM across the whole loop via matmul accumulation;
  adds are expressed as identity-matmuls on the TensorEngine (fp32r = TF32,
  full rate); ACT squares p straight out of PSUM; DVE does only the two final
  bias adds per d-tile.
"""

import numpy as np

import concourse.bacc as bacc
import concourse.mybir as mybir
import concourse.tile as tile
from concourse.bass_utils import run_bass_kernel_spmd

DT = 0.01
B, D, R = 4096, 1024, 256
NCORES = 8
BL = B // NCORES          # 512 batch columns per core
P = 128                   # partition dim
ND = D // P               # 8 d-tiles
NR = R // P               # 2 r-tiles
F32 = mybir.dt.float32
F32R = mybir.dt.float32r  # TF32 matmul mode: full PE rate


def _r(ap):
    return ap


def _emit(ctx, tc, steps, dram):
    nc = tc.nc
    n_id = 2 + max(steps - 1, 0)  # identity blocks: I, dt*I, T-weights

    sb = ctx.enter_context(tc.tile_pool(name="sb", bufs=1))
    qp = ctx.enter_context(tc.tile_pool(name="qp", bufs=2))
    pp = ctx.enter_context(tc.tile_pool(name="pp", bufs=1, space="PSUM"))
    ob = ctx.enter_context(tc.tile_pool(name="ob", bufs=4))

    # ---- load inputs to SBUF ----
    def load(name, shape, n, tagp, dt_=F32R):
        ts = []
        for i in range(n):
            t = sb.tile(shape, dt_, tag=f"{tagp}{i}", name=f"{tagp}{i}")
            nc.sync.dma_start(t[:], dram[name][i * P:(i + 1) * P, :])
            ts.append(t)
        return ts

    u_sb = load("Umat", [P, R], ND, "U")
    v_sb = load("vT", [P, BL], ND, "v")
    hn_sb = load("Hneg", [P, R], NR, "H")
    f_sb = load("fT", [P, BL], ND, "f")
    id_sb = sb.tile([P, n_id * P], F32R, tag="idp", name="idp_sb")
    nc.sync.dma_start(id_sb[:], dram["idp"][:])
    wn_sb = load("Wneg", [P, D], NR, "W")
    ub_sb = load("ub", [P, BL], ND, "ub", dt_=F32)
    xb_sb = load("xb", [P, BL], ND, "xb", dt_=F32)

    def idblk(i):
        return id_sb[:, i * P:(i + 1) * P]

    # ---- init: p = U^T v, c = U^T F (unscaled; dt folded into the dt*I add) ----
    p_ps = [pp.tile([P, BL], F32, tag=f"p{j}", name=f"p_ps{j}") for j in range(NR)]
    c_sb = [sb.tile([P, BL], F32R, tag=f"csb{j}", name=f"c_sb{j}") for j in range(NR)]
    with tc.tile_pool(name="cp", bufs=1, space="PSUM") as cp:
        c_ps = [cp.tile([P, BL], F32, tag=f"c{j}", name=f"c_ps{j}") for j in range(NR)]
        for j in range(NR):
            for d in range(ND):
                nc.tensor.matmul(
                    p_ps[j][:], _r(u_sb[d][:, j * P:(j + 1) * P]), _r(v_sb[d][:]),
                    start=(d == 0), stop=(d == ND - 1 and steps == 1),
                    skip_group_check=True,
                )
            for d in range(ND):
                nc.tensor.matmul(
                    c_ps[j][:], _r(u_sb[d][:, j * P:(j + 1) * P]), _r(f_sb[d][:]),
                    start=(d == 0), stop=(d == ND - 1),
                )
            nc.vector.tensor_copy(c_sb[j][:], c_ps[j][:])

    s_ps = [pp.tile([P, BL], F32, tag=f"S{j}", name=f"s_ps{j}") for j in range(NR)]
    t_ps = [pp.tile([P, BL], F32, tag=f"T{j}", name=f"t_ps{j}") for j in range(NR)] if steps > 1 else None

    # ---- the scan, entirely in R-space ----
    for k in range(steps):
        q = []
        for j in range(NR):
            qt = qp.tile([P, BL], F32R, tag=f"q{j}", name=f"q{j}_{k}")
            nc.scalar.activation(qt[:], p_ps[j][:], mybir.ActivationFunctionType.Square)
            q.append(qt)
        for j in range(NR):
            nc.tensor.matmul(
                s_ps[j][:], _r(idblk(0)), _r(q[j][:]),
                start=(k == 0), stop=(k == steps - 1),
            )
        if k < steps - 1:
            for j in range(NR):
                nc.tensor.matmul(
                    t_ps[j][:], _r(idblk(2 + k)), _r(q[j][:]),
                    start=(k == 0), stop=(k == steps - 2),
                )
            # p += dt*c - q @ H
            for j in range(NR):
                nc.tensor.matmul(
                    p_ps[j][:], _r(idblk(1)), _r(c_sb[j][:]),
                    start=False, stop=False, skip_group_check=True,
                )
                for r in range(NR):
                    nc.tensor.matmul(
                        p_ps[j][:], _r(hn_sb[r][:, j * P:(j + 1) * P]), _r(q[r][:]),
                        start=False, stop=(k == steps - 2 and r == NR - 1),
                        skip_group_check=True,
                    )

    s_sb = [sb.tile([P, BL], F32R, tag=f"Ss{j}", name=f"s_sb{j}") for j in range(NR)]
    for j in range(NR):
        nc.vector.tensor_copy(s_sb[j][:], s_ps[j][:])
    if steps > 1:
        t_sb = [sb.tile([P, BL], F32R, tag=f"Ts{j}", name=f"t_sb{j}") for j in range(NR)]
        for j in range(NR):
            nc.vector.tensor_copy(t_sb[j][:], t_ps[j][:])

    # ---- project back to D-space and add biases ----
    # x first: T closes one step before S, so its matmuls can start earlier.
    with tc.tile_pool(name="op", bufs=2, space="PSUM") as op:
        if steps > 1:
            for d in range(ND):
                o = op.tile([P, BL], F32, tag="o", name=f"ox{d}")
                for r in range(NR):
                    nc.tensor.matmul(
                        o[:], _r(wn_sb[r][:, d * P:(d + 1) * P]), _r(t_sb[r][:]),
                        start=(r == 0), stop=(r == NR - 1),
                    )
                res = ob.tile([P, BL], F32, tag="rx", name=f"rx{d}")
                nc.vector.tensor_add(res[:], xb_sb[d][:], o[:])
                nc.sync.dma_start(dram["cx"][d * P:(d + 1) * P, :], res[:])
        else:
            for d in range(ND):
                res = ob.tile([P, BL], F32, tag="rx", name=f"rx{d}")
                nc.vector.tensor_copy(res[:], xb_sb[d][:])
                nc.sync.dma_start(dram["cx"][d * P:(d + 1) * P, :], res[:])

        for d in range(ND):
            o = op.tile([P, BL], F32, tag="o", name=f"ov{d}")
            for r in range(NR):
                nc.tensor.matmul(
                    o[:], _r(wn_sb[r][:, d * P:(d + 1) * P]), _r(s_sb[r][:]),
                    start=(r == 0), stop=(r == NR - 1),
                )
            res = ob.tile([P, BL], F32, tag="rv", name=f"rv{d}")
            nc.vector.tensor_add(res[:], ub_sb[d][:], o[:])
            nc.sync.dma_start(dram["cv"][d * P:(d + 1) * P, :], res[:])


def _build(steps):
    from contextlib import ExitStack

    nc = bacc.Bacc("TRN2", target_bir_lowering=False, debug=False)
    n_id = 2 + max(steps - 1, 0)
    dram = {}
    for name, shape, dt_ in [
        ("vT", [D, BL], F32R), ("fT", [D, BL], F32R),
        ("ub", [D, BL], F32), ("xb", [D, BL], F32),
        ("Umat", [D, R], F32R), ("Wneg", [R, D], F32R), ("Hneg", [R, R], F32R),
        ("idp", [P, n_id * P], F32R),
    ]:
        dram[name] = nc.dram_tensor(name, shape, dt_, kind="ExternalInput").ap()
    for name in ["cv", "cx"]:
        dram[name] = nc.dram_tensor(name, [D, BL], F32, kind="ExternalOutput").ap()

    with tile.TileContext(nc) as tc:
        with ExitStack() as ctx:
            _emit(ctx, tc, steps, dram)
    nc.compile()
    return nc


_NC_CACHE = {}
TRACE = False
LAST_RESULT = None


def kernel(x, v, force, U, W, steps):
    global LAST_RESULT
    steps = int(np.asarray(steps))
    x = np.asarray(x, np.float32)
    v = np.asarray(v, np.float32)
    force = np.asarray(force, np.float32)
    U = np.asarray(U, np.float32)
    W = np.asarray(W, np.float32)
    if steps == 0:
        return x.copy(), v.copy()

    dt = np.float32(DT)
    c2 = np.float32(steps * (steps - 1) / 2.0)
    ub = v + (steps * dt) * force
    xb = x + (steps * dt) * v + (c2 * dt * dt) * force
    wneg = np.ascontiguousarray(-dt * W)
    hneg = np.ascontiguousarray(
        (-DT * (W.astype(np.float64) @ U.astype(np.float64))).astype(np.float32)
    )
    n_id = 2 + max(steps - 1, 0)
    idp = np.zeros((P, n_id * P), np.float32)
    eye = np.eye(P, dtype=np.float32)
    idp[:, 0:P] = eye
    idp[:, P:2 * P] = dt * eye
    for k in range(max(steps - 1, 0)):
        idp[:, (2 + k) * P:(3 + k) * P] = np.float32(DT * (steps - 1 - k)) * eye

    if steps not in _NC_CACHE:
        _NC_CACHE[steps] = _build(steps)
    nc = _NC_CACHE[steps]

    in_maps = []
    for c in range(NCORES):
        sl = slice(c * BL, (c + 1) * BL)
        in_maps.append({
            "vT": np.ascontiguousarray(v[sl].T),
            "fT": np.ascontiguousarray(force[sl].T),
            "ub": np.ascontiguousarray(ub[sl].T),
            "xb": np.ascontiguousarray(xb[sl].T),
            "Umat": U, "Wneg": wneg, "Hneg": hneg, "idp": idp,
        })

    res = run_bass_kernel_spmd(nc, in_maps, list(range(NCORES)), trace=TRACE)
    LAST_RESULT = res

    cx = np.empty((B, D), np.float32)
    cv = np.empty((B, D), np.float32)
    for c in range(NCORES):
        sl = slice(c * BL, (c + 1) * BL)
        cx[sl] = res.results[c]["cx"].T
        cv[sl] = res.results[c]["cv"].T
    return cx, cv
